# revision 1
# baseline (speedup 1.0000x reference)
"""CrossAttention (RoPE, 16 heads, C=1024) Trainium2 Bass kernel.

Sharding: DP over batch (4) x TP over heads (2 groups of 8) = 8 cores.
Each core computes, for its (batch b, head-group g):
  Q/K/V projections (column-parallel), RoPE, scores^T, exp (softmax without
  max-subtraction; logits are bounded), PV with an appended ones-column for
  the row-sums, late normalization, and the row-parallel output projection
  producing a partial out^T.  The host sums the two head-group partials.

All matmuls run as float32r (reduced-mantissa fp32) at full PE rate.

Pipeline: A (Q proj+RoPE) -> B (K proj+RoPE) -> pass0 (V proj fused with
attention for heads 0-2, key-chunk-outer, PE-bound) -> pass1 (attention for
heads 3-7, head-outer, exp-bound) -> E (output projection).

Layout notes (per core):
  qT  [C, Nq]   kT [C, Nk]   vT [C, Nk]      (activations, transposed on host)
  wqT/wkT/wvT [C, 512]   wpT [CH, C]          (weight slices, host-prepped)
  q/k rope tables [128, N] (64-row table duplicated; rows r use inv_freq[r%32])
  rT [128, 128]  block-diag rotate-half matrix:  rot(x) = rT.T @ x
  RoPE identity used:  rope(x) = x*cos + rT.T @ (x*sin)   (sin is 32-periodic
  along d, and rotate-half is a signed permutation within mod-32 classes).
"""

import sys

if "/opt/trn_rl_repo" not in sys.path:
    sys.path.insert(0, "/opt/trn_rl_repo")

import numpy as np
from contextlib import ExitStack

import concourse.bass as bass
import concourse.tile as tile
from concourse import bacc, mybir

F32 = mybir.dt.float32
F32R = mybir.dt.float32r
EXP = mybir.ActivationFunctionType.Exp

# problem constants
B, Nq, Nk, C = 4, 512, 2048, 1024
H, D = 16, 64
HL = 8            # heads per core
CH = HL * D       # 512 local channels
NPAIR = HL // 2   # 4 pair-chunks of 128 output dims
SC = Nk // 128    # 16 key chunks of 128
SB = Nk // 512    # 4 key blocks of 512
ROPE_BASE = 10000.0
SCALE = float(D) ** -0.5

P0H = 3                      # heads fused with the V projection in pass0
DG = [3, 3, 3, 3, 3, 1]      # pass1 exp grouping (s-chunks per PSUM tile)


def _ld3(nc, dst, src_2d, width=512):
    """One DMA loading a [N*128, width] DRAM region into a [128, N*width]
    tile (row-chunk ci lands at columns [ci*width, (ci+1)*width))."""
    nc.sync.dma_start(
        dst[:].rearrange("p (a s) -> p a s", s=width),
        src_2d.rearrange("(a p) s -> p a s", p=128))


def build_nc(iters: int = 1):
    nc = bacc.Bacc("TRN2", target_bir_lowering=False, debug=False)

    qT = nc.dram_tensor("qT", [C, Nq], F32, kind="ExternalInput")
    kT = nc.dram_tensor("kT", [C, Nk], F32, kind="ExternalInput")
    vT = nc.dram_tensor("vT", [C, Nk], F32, kind="ExternalInput")
    wqT = nc.dram_tensor("wqT", [C, CH], F32, kind="ExternalInput")
    wkT = nc.dram_tensor("wkT", [C, CH], F32, kind="ExternalInput")
    wvT = nc.dram_tensor("wvT", [C, CH], F32, kind="ExternalInput")
    wpT = nc.dram_tensor("wpT", [CH, C], F32, kind="ExternalInput")
    bpT = nc.dram_tensor("bpT", [128, 8], F32, kind="ExternalInput")
    qcos = nc.dram_tensor("qcos", [128, Nq], F32, kind="ExternalInput")
    qsin = nc.dram_tensor("qsin", [128, Nq], F32, kind="ExternalInput")
    kcos = nc.dram_tensor("kcos", [128, Nk], F32, kind="ExternalInput")
    ksin = nc.dram_tensor("ksin", [128, Nk], F32, kind="ExternalInput")
    rT = nc.dram_tensor("rT", [128, 128], F32, kind="ExternalInput")
    outT = nc.dram_tensor("outT", [C, Nq], F32, kind="ExternalOutput")

    def head_slices(h):
        """kr/qr pair index and row offset for local head h."""
        return h // 2, 64 * (h % 2)

    with tile.TileContext(nc) as tc, ExitStack() as top:
        const = top.enter_context(tc.tile_pool(name="const", bufs=1))
        rt_t = const.tile([128, 128], F32R, tag="rt", name="rt")
        nc.sync.dma_start(rt_t[:], rT[:].bitcast(F32R))
        bp_t = const.tile([128, 8], F32, tag="bp", name="bp")
        nc.sync.dma_start(bp_t[:], bpT[:])
        ones_f32 = const.tile([128, 128], F32, tag="ones_f32", name="ones_f32")
        nc.vector.memset(ones_f32[:], 1.0)
        ones_t = const.tile([128, 64], F32R, tag="ones", name="ones")
        nc.vector.tensor_copy(ones_t[:], ones_f32[:, 0:64])

        for _ in range(iters):
            with ExitStack() as it_stack:
                qkr = it_stack.enter_context(tc.tile_pool(name="qkr", bufs=1))
                qr_t = [qkr.tile([128, Nq], F32R, tag=f"qr{m}", name=f"qr{m}")
                        for m in range(NPAIR)]
                kr_t = [qkr.tile([128, Nk], F32R, tag=f"kr{m}", name=f"kr{m}")
                        for m in range(NPAIR)]
                vxt = it_stack.enter_context(tc.tile_pool(name="vxt", bufs=1))
                v65 = vxt.tile([128, SC * 520], F32R, tag="v65", name="v65")
                xt_t = [vxt.tile([65, Nq], F32R, tag=f"xt{h}", name=f"xt{h}")
                        for h in range(HL)]
                # attention-prob tiles + normalization scratch span pass0+pass1
                dpool = it_stack.enter_context(tc.tile_pool(name="dpool",
                                                            bufs=1))

                def exp_tile(width):
                    return dpool.tile([128, width], F32R, tag="pt", name="pt",
                                      bufs=2, padded_shape=[128, 1536])

                def normalize(h):
                    """x = x / rowsum, in place on xt rows 0:64 (pr from the
                    caller's PSUM pool via matmul against the ones row)."""
                    inv = dpool.tile([64, Nq], F32, tag="inv", name="inv",
                                     bufs=2)
                    return inv

                # ========== Phases A/B: Q/K projections + RoPE ==========
                with ExitStack() as ab:
                    stream = ab.enter_context(
                        tc.tile_pool(name="stream", bufs=2))
                    wstage = ab.enter_context(
                        tc.tile_pool(name="wstage", bufs=2))

                    with ExitStack() as abx:
                        ppsum = abx.enter_context(
                            tc.tile_pool(name="ppsum", bufs=2, space="PSUM"))
                        rpsum = abx.enter_context(
                            tc.tile_pool(name="rpsum", bufs=2, space="PSUM"))
                        ktbl = abx.enter_context(
                            tc.tile_pool(name="ktbl", bufs=2))
                        rope = abx.enter_context(
                            tc.tile_pool(name="rope", bufs=1))
                        evac = abx.enter_context(
                            tc.tile_pool(name="evac", bufs=2))

                        def rope_block(x_psum, cos_ap, sin_ap, out_ap, width):
                            """out = x*cos + rT.T @ (x*sin); ACT evacuates."""
                            xs = evac.tile([128, width], F32, tag="xs",
                                           name="xs")
                            nc.scalar.copy(xs[:], x_psum[:])
                            tsin = rope.tile([128, width], F32R, tag="tsin",
                                             name="tsin")
                            nc.vector.tensor_mul(tsin[:], xs[:], sin_ap)
                            prot = rpsum.tile([128, width], F32, tag="prot",
                                              name="prot")
                            nc.tensor.matmul(prot[:], rt_t[:], tsin[:],
                                             start=True, stop=True)
                            tcos = rope.tile([128, width], F32, tag="tcos",
                                             name="tcos")
                            nc.vector.tensor_mul(tcos[:], xs[:], cos_ap)
                            nc.vector.tensor_add(out_ap, tcos[:], prot[:])

                        # ---- A: Q projection + RoPE ----
                        qc_t = ktbl.tile([128, Nq], F32, tag="kcos",
                                         name="qcos")
                        nc.sync.dma_start(qc_t[:], qcos[:])
                        qs_t = ktbl.tile([128, Nq], F32, tag="ksin",
                                         name="qsin")
                        nc.sync.dma_start(qs_t[:], qsin[:])
                        wq_t = wstage.tile([128, 8 * CH], F32R, tag="w",
                                           name="wq")
                        _ld3(nc, wq_t, wqT[:].bitcast(F32R), CH)
                        qt_t = stream.tile([128, 4096], F32R, tag="s",
                                           name="qt")
                        _ld3(nc, qt_t, qT[:].bitcast(F32R))

                        for m in range(NPAIR):
                            pq = ppsum.tile([128, Nq], F32, tag="pq",
                                            name="pq")
                            for ci in range(8):
                                nc.tensor.matmul(
                                    pq[:],
                                    wq_t[:, ci * CH + m * 128:
                                         ci * CH + (m + 1) * 128],
                                    qt_t[:, ci * 512:(ci + 1) * 512],
                                    start=(ci == 0), stop=(ci == 7))
                            rope_block(pq, qc_t[:], qs_t[:], qr_t[m][:], Nq)

                        # ---- B: K projection + RoPE ----
                        wk_t = wstage.tile([128, 8 * CH], F32R, tag="w",
                                           name="wk")
                        _ld3(nc, wk_t, wkT[:].bitcast(F32R), CH)
                        for sbi in range(SB):
                            sl = slice(sbi * 512, (sbi + 1) * 512)
                            kc_t = ktbl.tile([128, 512], F32, tag="kcos",
                                             name="kcos")
                            nc.sync.dma_start(kc_t[:], kcos[:, sl])
                            ks_t = ktbl.tile([128, 512], F32, tag="ksin",
                                             name="ksin")
                            nc.sync.dma_start(ks_t[:], ksin[:, sl])
                            kt_t = stream.tile([128, 4096], F32R, tag="s",
                                               name="kt")
                            _ld3(nc, kt_t, kT[:, sl].bitcast(F32R))
                            for m in range(NPAIR):
                                pk = ppsum.tile([128, 512], F32, tag="pq",
                                                name="pk")
                                for ci in range(8):
                                    nc.tensor.matmul(
                                        pk[:],
                                        wk_t[:, ci * CH + m * 128:
                                             ci * CH + (m + 1) * 128],
                                        kt_t[:, ci * 512:(ci + 1) * 512],
                                        start=(ci == 0), stop=(ci == 7))
                                rope_block(pk, kc_t[:], ks_t[:],
                                           kr_t[m][:, sl], 512)

                    # ===== pass0: V projection + attention heads 0..P0H-1,
                    # key-chunk-outer (wstage/stream stay open for wv/vT) ====
                    ones_cols = v65[:].rearrange(
                        "p (n w) -> p n w", w=65)[:, :, 64:65]
                    nc.vector.tensor_copy(
                        ones_cols,
                        ones_f32[:, 0:SC * 8].rearrange(
                            "p (n w) -> p n w", w=1))
                    with ExitStack() as ph:
                        pv_pool = ph.enter_context(
                            tc.tile_pool(name="pv0", bufs=2, space="PSUM"))
                        psc_pool = ph.enter_context(
                            tc.tile_pool(name="psc0", bufs=1, space="PSUM"))
                        pxt_pool = ph.enter_context(
                            tc.tile_pool(name="pxt0", bufs=1, space="PSUM"))

                        wv_t = wstage.tile([128, 8 * CH], F32R, tag="w",
                                           name="wv")
                        _ld3(nc, wv_t, wvT[:].bitcast(F32R), CH)
                        pxt0 = [pxt_pool.tile([65, Nq], F32, tag=f"px{h}",
                                              name=f"px{h}")
                                for h in range(P0H)]
                        for sbi in range(SB):
                            sl = slice(sbi * 512, (sbi + 1) * 512)
                            vt_t = stream.tile([128, 4096], F32R, tag="s",
                                               name="vt")
                            _ld3(nc, vt_t, vT[:, sl].bitcast(F32R))
                            for scj in range(4):
                                sc = sbi * 4 + scj
                                pv = pv_pool.tile([128, CH], F32, tag="pv",
                                                  name="pv")
                                for ci in range(8):
                                    nc.tensor.matmul(
                                        pv[:],
                                        vt_t[:, ci * 512 + scj * 128:
                                             ci * 512 + (scj + 1) * 128],
                                        wv_t[:, ci * CH:(ci + 1) * CH],
                                        start=(ci == 0), stop=(ci == 7))
                                dst = v65[:, sc * 520:(sc + 1) * 520
                                          ].rearrange(
                                              "p (n w) -> p n w",
                                              w=65)[:, :, 0:64]
                                nc.scalar.copy(
                                    dst,
                                    pv[:].rearrange("p (n w) -> p n w", w=64))

                                psc = psc_pool.tile([128, 512 * P0H], F32,
                                                    tag="psc", name="psc")
                                for hj in range(P0H):
                                    p, r0 = head_slices(hj)
                                    nc.tensor.matmul(
                                        psc[:, hj * 512:(hj + 1) * 512],
                                        kr_t[p][r0:r0 + 64,
                                                sc * 128:(sc + 1) * 128],
                                        qr_t[p][r0:r0 + 64, :],
                                        start=True, stop=True)
                                pt = exp_tile(512 * P0H)
                                nc.scalar.activation(pt[:], psc[:], EXP,
                                                     scale=SCALE)
                                for hj in range(P0H):
                                    nc.tensor.matmul(
                                        pxt0[hj][:],
                                        v65[:, sc * 520 + hj * 65:
                                            sc * 520 + hj * 65 + 65],
                                        pt[:, hj * 512:(hj + 1) * 512],
                                        start=(sc == 0), stop=(sc == SC - 1),
                                        skip_group_check=True)
                        for hj in range(P0H):
                            nc.vector.tensor_copy(xt_t[hj][:], pxt0[hj][:])

                # ===== pass1: attention heads P0H..7, head-outer =====
                with ExitStack() as phd:
                    wp_pool = phd.enter_context(tc.tile_pool(name="wpp",
                                                             bufs=1))
                    wp_t = wp_pool.tile([64, HL * C], F32R, tag="wp",
                                        name="wp")
                    nc.sync.dma_start(
                        wp_t[:].rearrange("p (a s) -> p a s", s=C),
                        wpT[:].bitcast(F32R).rearrange("(a p) s -> p a s",
                                                       p=64))

                    with ExitStack() as ph:
                        psc_pool = ph.enter_context(
                            tc.tile_pool(name="psc1", bufs=2, space="PSUM"))
                        pxt_pool = ph.enter_context(
                            tc.tile_pool(name="pxt1", bufs=2, space="PSUM"))

                        def do_normalize(h):
                            pr = pxt_pool.tile([64, Nq], F32, tag="pxt",
                                               name="pr")
                            nc.tensor.matmul(pr[:], ones_t[64:65, :],
                                             xt_t[h][64:65, :],
                                             start=True, stop=True)
                            inv = normalize(h)
                            nc.vector.reciprocal(inv[:], pr[:])
                            nc.vector.tensor_mul(
                                xt_t[h][0:64, :],
                                xt_t[h][0:64, :].bitcast(F32), inv[:])

                        for hj in range(P0H):
                            do_normalize(hj)

                        for h in range(P0H, HL):
                            p, r0 = head_slices(h)
                            pxt = pxt_pool.tile([65, Nq], F32, tag="pxt",
                                                name="pxt")
                            sc0 = 0
                            for gw in DG:
                                psc = psc_pool.tile([128, 512 * gw], F32,
                                                    tag="psc", name="psc")
                                for j in range(gw):
                                    sc = sc0 + j
                                    nc.tensor.matmul(
                                        psc[:, j * 512:(j + 1) * 512],
                                        kr_t[p][r0:r0 + 64,
                                                sc * 128:(sc + 1) * 128],
                                        qr_t[p][r0:r0 + 64, :],
                                        start=True, stop=True)
                                pt = exp_tile(512 * gw)
                                nc.scalar.activation(pt[:], psc[:], EXP,
                                                     scale=SCALE)
                                for j in range(gw):
                                    sc = sc0 + j
                                    nc.tensor.matmul(
                                        pxt[:],
                                        v65[:, sc * 520 + h * 65:
                                            sc * 520 + h * 65 + 65],
                                        pt[:, j * 512:(j + 1) * 512],
                                        start=(sc == 0), stop=(sc == SC - 1),
                                        skip_group_check=True)
                                sc0 += gw
                            nc.vector.tensor_copy(xt_t[h][:], pxt[:])
                            do_normalize(h)

                    # ========== E: output projection ==========
                    with ExitStack() as ph:
                        pool = ph.enter_context(tc.tile_pool(name="phE",
                                                             bufs=3))
                        po_pool = ph.enter_context(
                            tc.tile_pool(name="poE", bufs=4, space="PSUM"))

                        for j in range(8):
                            po = po_pool.tile([128, Nq], F32, tag="po",
                                              name="po")
                            for h in range(HL):
                                nc.tensor.matmul(
                                    po[:],
                                    wp_t[:, h * C + j * 128:
                                         h * C + (j + 1) * 128],
                                    xt_t[h][0:64, :], start=(h == 0),
                                    stop=(h == 7))
                            osb = pool.tile([128, Nq], F32, tag="osb",
                                            name="osb")
                            nc.vector.tensor_scalar_add(osb[:], po[:],
                                                        bp_t[:, j:j + 1])
                            nc.sync.dma_start(
                                outT[j * 128:(j + 1) * 128, :], osb[:])

    nc.compile()
    return nc


def prep_inputs(query, key, value, qpos, kpos, Wq, Wk, Wv, Wp, bp):
    """Build per-core input maps (8 cores: core = 2*b + g)."""
    invf = (1.0 / ROPE_BASE ** (np.arange(0, D, 2, dtype=np.float32) / D)
            ).astype(np.float32)
    rows64 = invf[np.arange(64) % 32]          # [64]

    R64 = np.zeros((64, 64), dtype=np.float32)
    for r in range(32):
        R64[r, r + 32] = -1.0
        R64[r + 32, r] = 1.0
    rT128 = np.zeros((128, 128), dtype=np.float32)
    rT128[0:64, 0:64] = R64.T
    rT128[64:128, 64:128] = R64.T

    in_maps = []
    for core in range(8):
        b, g = core // 2, core % 2
        cols = slice(g * CH, (g + 1) * CH)
        qang = rows64[:, None] * np.asarray(qpos[b], np.float32)[None, :]
        kang = rows64[:, None] * np.asarray(kpos[b], np.float32)[None, :]
        m = {
            "qT": np.ascontiguousarray(np.asarray(query[b], np.float32).T),
            "kT": np.ascontiguousarray(np.asarray(key[b], np.float32).T),
            "vT": np.ascontiguousarray(np.asarray(value[b], np.float32).T),
            "wqT": np.ascontiguousarray(np.asarray(Wq, np.float32)[cols, :].T),
            "wkT": np.ascontiguousarray(np.asarray(Wk, np.float32)[cols, :].T),
            "wvT": np.ascontiguousarray(np.asarray(Wv, np.float32)[cols, :].T),
            "wpT": np.ascontiguousarray(np.asarray(Wp, np.float32)[:, cols].T),
            "bpT": (np.ascontiguousarray(
                        np.asarray(bp, np.float32).reshape(8, 128).T)
                    if g == 0 else np.zeros((128, 8), np.float32)),
            "qcos": np.ascontiguousarray(
                np.tile(np.cos(qang), (2, 1)).astype(np.float32)),
            "qsin": np.ascontiguousarray(
                np.tile(np.sin(qang), (2, 1)).astype(np.float32)),
            "kcos": np.ascontiguousarray(
                np.tile(np.cos(kang), (2, 1)).astype(np.float32)),
            "ksin": np.ascontiguousarray(
                np.tile(np.sin(kang), (2, 1)).astype(np.float32)),
            "rT": rT128,
        }
        in_maps.append(m)
    return in_maps


_NC_CACHE = {}


def _get_nc(iters=1):
    if iters not in _NC_CACHE:
        _NC_CACHE[iters] = build_nc(iters)
    return _NC_CACHE[iters]


def kernel(query, key, value, qpos, kpos, Wq, Wk, Wv, Wp, bp):
    from concourse.bass_utils import run_bass_kernel_spmd

    nc = _get_nc()
    in_maps = prep_inputs(query, key, value, qpos, kpos, Wq, Wk, Wv, Wp, bp)
    res = run_bass_kernel_spmd(nc, in_maps, list(range(8)))
    out = np.zeros((B, Nq, C), dtype=np.float32)
    for core in range(8):
        out[core // 2] += res.results[core]["outT"].T
    return out

